# revision 11
# baseline (speedup 1.0000x reference)
"""TRN2 Bass kernel for nn_ActionableRGM: z_t = S @ T_irrep(x_t) @ S^-1 @ z_{t-1}.

Closed form: in the S^-1 basis the transfer matrix is block-diagonal 2x2
rotations, and rotations compose by adding angles, so the L-sequential scan
collapses to a prefix sum of per-step angles:

    kx[b,l,m] = om[m,:] @ x[b,l,:]            (angles per step, in turns)
    K = cumsum_l(kx)
    w0 = S^-1 z0,  a = w0[odd], bb = w0[even]
    out[b,l,:] = cos(2pi K) @ U.T + sin(2pi K) @ V.T + w0[0]*S[:,0]
      where U = a*S[:,odd] + bb*S[:,even],  V = a*S[:,even] - bb*S[:,odd]

Sharding: data-parallel over B (16 batches / 8 cores = 2 per core).

Device pipeline per core (128 partitions = 2 batches x 64 freq pairs; no
cross-partition data movement anywhere):
  PE:   kx via block-diag lhsT (fp32 exact, split in halves to overlap the
        scan), then per (batch, l-chunk) cos/sin accumulating matmuls,
        contract=64, fp32r with rhs padded to N=256 (1 cycle/row when warm).
  DVE:  tensor_tensor_scan cumsum along L (two chained halves), magic-const
        rounding, frac = ks - round(ks), psum eviction fused with +const row.
  ACT:  Abs, cos = Sin(-2pi|f| + pi/2), sin = Sin(2pi f) on full 128-row
        tiles; a dummy Sin up front overlaps the one-time table load.
  DMA:  two packed param loads; four half-batch output DMAs split across
        the sync and scalar HWDGE rings.

build_nc(reps=K) unrolls the pipeline K times back-to-back inside one NEFF:
wall-clock deltas between rep counts give dispatch-overhead-free HW timing.
"""
import sys

sys.path.insert(0, "/opt/trn_rl_repo")

import numpy as np
import concourse.bass as bass
import concourse.mybir as mybir
from concourse.tile import TileContext
from concourse.vector_clock import ScopedClock

B, L, D = 16, 1024, 129
M = 64
N_CORES = 8
BPC = B // N_CORES
LROWS = BPC * L
N_CHUNKS = L // 128
DP = 256                     # rhs free dim padded so fp32r runs 1 cycle/row
WX = L + 128                 # packed xom width: xT(1024) ++ omblk(128)
WU = 2 * DP + 2 * D          # packed uvc: UTx2(256) ++ VTx2(256) ++ crow x2

C_MAGIC = 12582912.0         # 1.5 * 2**23: fp32 round-to-nearest-int magic
TWO_PI = float(2.0 * np.pi)
HALF_PI = float(np.pi / 2.0)
F32 = mybir.dt.float32
F32R = mybir.dt.float32r
BF16 = mybir.dt.bfloat16
AF = mybir.ActivationFunctionType
OP = mybir.AluOpType


class PatchedTileContext(TileContext):
    """Work around walrus rejecting >1 sync-wait on the final kernel drain
    ("Too many sync wait commands" in CoreV3GenImpl::setupSyncWait): split
    the global-clock waits across a chain of drains, one wait each."""

    def _drain_and_barrier(self, tick_clock, wait_clock):
        drain_inst = self.nc.sync.drain()
        wait_clock.add_sem_waits(
            drain_inst.ins, ScopedClock({None: tick_clock.global_clock})
        )
        si = drain_inst.ins.sync_info
        waits = list(si.on_wait) if (si and si.on_wait) else []
        if len(waits) > 1:
            si.on_wait = waits[:1]
            for w in waits[1:]:
                d2 = self.nc.sync.drain()
                d2.ins.sync_info = mybir.SyncInfo(on_wait=[w], on_update=[])
        self.nc.all_engine_barrier()
        assert self.sems is not None
        popped = self.nc._tile_sem_poison_stack.pop()
        assert popped is self._sem_poison
        self.nc.clear_and_free_semaphores(list(self.sems.allocated().values()))
        self.nc.all_engine_barrier()


_WFIX_CTR = [0]


def _split_multiwait(nc, max_waits=1):
    """This walrus build rejects instructions carrying more than ~1 sync
    wait ("Too many sync wait commands"). Hoist excess waits onto no-op
    instructions inserted just before the offender on the same engine —
    waits only move earlier, so this is always sound."""
    for f in nc.m.functions:
        for blk in f.blocks:
            out, changed = [], False
            for inst in blk.instructions:
                si = inst.sync_info
                waits = list(si.on_wait) if (si and si.on_wait) else []
                if len(waits) > max_waits:
                    changed = True
                    excess = waits[:len(waits) - max_waits]
                    for k in range(0, len(excess), max_waits):
                        nop = mybir.InstNoOp(
                            name=f"wfix-{_WFIX_CTR[0]}", ins=[], outs=[])
                        _WFIX_CTR[0] += 1
                        nop.engine = inst.engine
                        nop.sync_info = mybir.SyncInfo(
                            on_wait=excess[k:k + max_waits], on_update=[])
                        out.append(nop)
                    si.on_wait = waits[len(waits) - max_waits:]
                out.append(inst)
            if changed:
                blk.instructions = out


def _emit_rep(nc, consts, pools, use_f32r, rep):
    (zero_bias, halfpi_bias, zeros_t, xo, uvc_t, out) = consts
    work, kxp, ops = pools
    mm_dt = F32R if use_f32r else F32
    nmm = DP if use_f32r else D

    xt = xo[:, 0:L]
    omb = xo[:, L:WX]
    rhs_cos = uvc_t[:, 0:nmm]
    rhs_sin = uvc_t[:, DP:DP + nmm]
    crowB = (uvc_t[:, 2 * DP:WU].bitcast(F32)
             .rearrange("p (j d) -> p j d", j=2))

    # ---- kx (turns) + prefix sum, pipelined in halves ----
    kxps = kxp.tile([128, L], F32, tag="kxps")
    ks = work.tile([128, L], F32, tag="ks")
    nc.tensor.matmul(kxps[:, 0:512], lhsT=omb, rhs=xt[:, 0:512],
                     start=True, stop=True)
    nc.vector.tensor_tensor_scan(ks[:, 0:512], kxps[:, 0:512], zeros_t,
                                 0.0, op0=OP.add, op1=OP.add)
    nc.tensor.matmul(kxps[:, 512:1024], lhsT=omb, rhs=xt[:, 512:1024],
                     start=True, stop=True)
    nc.vector.tensor_tensor_scan(ks[:, 512:1024], kxps[:, 512:1024],
                                 zeros_t, ks[:, 511:512],
                                 op0=OP.add, op1=OP.add)

    # ---- range reduction: frac = ks - round(ks) in [-0.5, 0.5] ----
    rnd = work.tile([128, L], F32, tag="rnd")
    nc.vector.tensor_scalar(rnd, ks, C_MAGIC, C_MAGIC,
                            op0=OP.add, op1=OP.subtract)
    frac = work.tile([128, L], F32, tag="frac")
    nc.vector.tensor_tensor(frac, ks, rnd, op=OP.subtract)
    af = work.tile([128, L], F32, tag="af")
    nc.scalar.activation(af, frac, AF.Abs, bias=zero_bias)

    # ---- trig: cos = sin(pi/2 - 2pi |f|), sin = sin(2pi f) ----
    mmdt = F32R if use_f32r else F32
    cosT = work.tile([128, L], mmdt, tag="cosT")
    nc.scalar.activation(cosT, af, AF.Sin, scale=-TWO_PI, bias=halfpi_bias)
    sinT = work.tile([128, L], mmdt, tag="sinT")
    nc.scalar.activation(sinT, frac, AF.Sin, scale=TWO_PI, bias=zero_bias)

    # ---- output matmuls + fused (+const) eviction + DMA ----
    for b in range(BPC):
        pp = slice(b * M, (b + 1) * M)
        for h in range(2):
            obuf = work.tile([128, N_CHUNKS // 2, D], F32, tag=f"obuf{b}{h}")
            for i in range(0, N_CHUNKS // 2, 2):
                ic = h * (N_CHUNKS // 2) + i
                ps = ops.tile([128, 2, nmm], F32, tag="ops")
                for j in (0, 1):
                    lsl = slice((ic + j) * 128, (ic + j + 1) * 128)
                    nc.tensor.matmul(
                        ps[:, j, :], lhsT=cosT[pp, lsl],
                        rhs=rhs_cos[pp, :], start=True, stop=False)
                    nc.tensor.matmul(
                        ps[:, j, :], lhsT=sinT[pp, lsl],
                        rhs=rhs_sin[pp, :], start=False, stop=True)
                nc.vector.scalar_tensor_tensor(
                    obuf[:, i:i + 2, :], ps[:, :, 0:D], 0.0, crowB,
                    op0=OP.add, op1=OP.add,
                )
            eng = nc.sync if b == 0 else nc.scalar
            r0 = b * L + h * (L // 2)
            eng.dma_start(
                out=out[r0:r0 + L // 2, :].rearrange("(i p) d -> p i d", p=128),
                in_=obuf,
            )


def build_nc(use_f32r=True, reps=1, wfix=True, loop_k=0):
    nc = bass.Bass()
    udt = F32R if use_f32r else F32
    xom = nc.declare_dram_parameter("xom", [2 * BPC, WX], F32, isOutput=False)
    uvc = nc.declare_dram_parameter("uvc", [128, WU], udt, isOutput=False)
    out = nc.declare_dram_parameter("out", [LROWS, D], F32, isOutput=True)

    with PatchedTileContext(nc) as tc:
        with (
            tc.tile_pool(name="singles", bufs=1) as singles,
            tc.tile_pool(name="work", bufs=2) as work,
            tc.tile_pool(name="kxp", bufs=2, space="PSUM") as kxp,
            tc.tile_pool(name="ops", bufs=4, space="PSUM") as ops,
        ):
            zero_bias = singles.tile([128, 1], F32)
            nc.gpsimd.memset(zero_bias, 0.0)
            halfpi_bias = singles.tile([128, 1], F32)
            nc.gpsimd.memset(halfpi_bias, HALF_PI)
            zeros_t = singles.tile([128, 512], F32)
            nc.gpsimd.memset(zeros_t, 0.0)
            dummy = singles.tile([128, 1], F32)
            nc.scalar.activation(dummy, zero_bias, AF.Sin,
                                 scale=1.0, bias=zero_bias)

            xo = singles.tile([2 * BPC, WX], F32)
            nc.sync.dma_start(out=xo, in_=xom[:, :])
            uvc_t = singles.tile([128, WU], udt)
            nc.sync.dma_start(out=uvc_t, in_=uvc[:, :])

            consts = (zero_bias, halfpi_bias, zeros_t, xo, uvc_t, out)
            pools = (work, kxp, ops)
            if loop_k:
                with tc.For_i(0, loop_k, 1) as _i:
                    _emit_rep(nc, consts, pools, use_f32r, 0)
            else:
                for rep in range(reps):
                    _emit_rep(nc, consts, pools, use_f32r, rep)
    if wfix:
        _split_multiwait(nc)
    return nc


def host_prep(input, z0, om, S):
    """Precompute the tiny parameter-derived tensors (all O(D*M))."""
    input = np.ascontiguousarray(input, dtype=np.float32)
    z0 = np.asarray(z0, dtype=np.float32)
    om = np.asarray(om, dtype=np.float32)
    S = np.asarray(S, dtype=np.float32)

    om_t = (om.astype(np.float64) / (2.0 * np.pi)).astype(np.float32)  # turns
    w0 = np.linalg.solve(S.astype(np.float64), z0.astype(np.float64))
    a = w0[1::2]
    bb = w0[2::2]
    Sd = S.astype(np.float64)
    U = (a * Sd[:, 1::2] + bb * Sd[:, 2::2])          # [D, M]
    V = (a * Sd[:, 2::2] - bb * Sd[:, 1::2])          # [D, M]
    crow = (w0[0] * Sd[:, 0]).astype(np.float32)

    uvcp = np.zeros((128, WU), dtype=np.float32)
    uvcp[0:M, 0:D] = U.T.astype(np.float32)
    uvcp[M:2 * M, 0:D] = U.T.astype(np.float32)
    uvcp[0:M, DP:DP + D] = V.T.astype(np.float32)
    uvcp[M:2 * M, DP:DP + D] = V.T.astype(np.float32)
    uvcp[:, 2 * DP:2 * DP + D] = crow
    uvcp[:, 2 * DP + D:WU] = crow

    omblk = np.zeros((2 * BPC, 128), dtype=np.float32)
    omblk[0:2, 0:M] = om_t.T
    omblk[2:4, M:2 * M] = om_t.T

    xoms = []
    for c in range(N_CORES):
        xc = input[c * BPC:(c + 1) * BPC]            # [2, L, 2]
        xo = np.zeros((2 * BPC, WX), dtype=np.float32)
        xo[:, 0:L] = xc.transpose(0, 2, 1).reshape(2 * BPC, L)
        xo[:, L:WX] = omblk
        xoms.append(xo)
    return xoms, uvcp


def kernel(input, z0, om, S):
    from concourse.bass_utils import run_bass_kernel_spmd

    xoms, uvcp = host_prep(input, z0, om, S)
    nc = build_nc()
    in_maps = [{"xom": xoms[c], "uvc": uvcp} for c in range(N_CORES)]
    res = run_bass_kernel_spmd(nc, in_maps, list(range(N_CORES)))
    outputs = np.empty((B, L, D), dtype=np.float32)
    for c in range(N_CORES):
        outputs[c * BPC:(c + 1) * BPC] = res.results[c]["out"].reshape(BPC, L, D)
    z_final = outputs[:, -1, :].copy()
    return outputs, z_final


# revision 12
# speedup vs baseline: 1.2821x; 1.2821x over previous
"""TRN2 Bass kernel for nn_ActionableRGM: z_t = S @ T_irrep(x_t) @ S^-1 @ z_{t-1}.

Closed form: in the S^-1 basis the transfer matrix is block-diagonal 2x2
rotations, and rotations compose by adding angles, so the L-sequential scan
collapses to a prefix sum of per-step angles:

    kx[b,l,m] = om[m,:] @ x[b,l,:]            (angles per step, in turns)
    K = cumsum_l(kx)
    w0 = S^-1 z0,  a = w0[odd], bb = w0[even]
    out[b,l,:] = cos(2pi K) @ U.T + sin(2pi K) @ V.T + w0[0]*S[:,0]
      where U = a*S[:,odd] + bb*S[:,even],  V = a*S[:,even] - bb*S[:,odd]

Sharding: data-parallel over B (16 batches / 8 cores = 2 per core).

Device pipeline per core (128 partitions = 2 batches x 64 freq pairs; no
cross-partition data movement anywhere):
  PE:   kx via block-diag lhsT (fp32 exact, split in halves to overlap the
        scan), then per (batch, l-chunk) cos/sin accumulating matmuls,
        contract=64, fp32r with rhs padded to N=256 (1 cycle/row when warm).
  DVE:  tensor_tensor_scan cumsum along L (two chained halves), magic-const
        rounding, frac = ks - round(ks), psum eviction fused with +const row.
  ACT:  Abs, cos = Sin(-2pi|f| + pi/2), sin = Sin(2pi f) on full 128-row
        tiles; a dummy Sin up front overlaps the one-time table load.
  DMA:  two packed param loads; four half-batch output DMAs split across
        the sync and scalar HWDGE rings.

build_nc(reps=K) unrolls the pipeline K times back-to-back inside one NEFF:
wall-clock deltas between rep counts give dispatch-overhead-free HW timing.
"""
import sys

sys.path.insert(0, "/opt/trn_rl_repo")

import numpy as np
import concourse.bass as bass
import concourse.mybir as mybir
from concourse.tile import TileContext
from concourse.vector_clock import ScopedClock

B, L, D = 16, 1024, 129
M = 64
N_CORES = 8
BPC = B // N_CORES
LROWS = BPC * L
N_CHUNKS = L // 128
DP = 256                     # rhs free dim padded so fp32r runs 1 cycle/row
WX = L + 128                 # packed xom width: xT(1024) ++ omblk(128)
WU = 2 * DP + 2 * D          # packed uvc: UTx2(256) ++ VTx2(256) ++ crow x2

C_MAGIC = 12582912.0         # 1.5 * 2**23: fp32 round-to-nearest-int magic
TWO_PI = float(2.0 * np.pi)
HALF_PI = float(np.pi / 2.0)
F32 = mybir.dt.float32
F32R = mybir.dt.float32r
BF16 = mybir.dt.bfloat16
AF = mybir.ActivationFunctionType
OP = mybir.AluOpType


class PatchedTileContext(TileContext):
    """Work around walrus rejecting >1 sync-wait on the final kernel drain
    ("Too many sync wait commands" in CoreV3GenImpl::setupSyncWait): split
    the global-clock waits across a chain of drains, one wait each."""

    def _drain_and_barrier(self, tick_clock, wait_clock):
        drain_inst = self.nc.sync.drain()
        wait_clock.add_sem_waits(
            drain_inst.ins, ScopedClock({None: tick_clock.global_clock})
        )
        si = drain_inst.ins.sync_info
        waits = list(si.on_wait) if (si and si.on_wait) else []
        if len(waits) > 1:
            si.on_wait = waits[:1]
            for w in waits[1:]:
                d2 = self.nc.sync.drain()
                d2.ins.sync_info = mybir.SyncInfo(on_wait=[w], on_update=[])
        self.nc.all_engine_barrier()
        assert self.sems is not None
        popped = self.nc._tile_sem_poison_stack.pop()
        assert popped is self._sem_poison
        self.nc.clear_and_free_semaphores(list(self.sems.allocated().values()))
        self.nc.all_engine_barrier()


_WFIX_CTR = [0]


def _split_multiwait(nc, max_waits=1):
    """This walrus build rejects instructions carrying more than ~1 sync
    wait ("Too many sync wait commands"). Hoist excess waits onto no-op
    instructions inserted just before the offender on the same engine —
    waits only move earlier, so this is always sound."""
    for f in nc.m.functions:
        for blk in f.blocks:
            out, changed = [], False
            for inst in blk.instructions:
                si = inst.sync_info
                waits = list(si.on_wait) if (si and si.on_wait) else []
                if len(waits) > max_waits:
                    changed = True
                    excess = waits[:len(waits) - max_waits]
                    for k in range(0, len(excess), max_waits):
                        nop = mybir.InstNoOp(
                            name=f"wfix-{_WFIX_CTR[0]}", ins=[], outs=[])
                        _WFIX_CTR[0] += 1
                        nop.engine = inst.engine
                        nop.sync_info = mybir.SyncInfo(
                            on_wait=excess[k:k + max_waits], on_update=[])
                        out.append(nop)
                    si.on_wait = waits[len(waits) - max_waits:]
                out.append(inst)
            if changed:
                blk.instructions = out


def _emit_rep(nc, consts, pools, use_f32r, rep, variant="full"):
    (zero_bias, halfpi_bias, zeros_t, xo, uvc_t, out) = consts
    work, kxp, ops = pools
    mm_dt = F32R if use_f32r else F32
    nmm = DP if use_f32r else D

    xt = xo[:, 0:L]
    omb = xo[:, L:WX]
    rhs_cos = uvc_t[:, 0:nmm]
    rhs_sin = uvc_t[:, DP:DP + nmm]
    crowB = (uvc_t[:, 2 * DP:WU].bitcast(F32)
             .rearrange("p (j d) -> p j d", j=2))

    if variant == "mmonly":
        cosT = work.tile([128, L], F32R if use_f32r else F32, tag="cosT")
        nc.vector.memset(cosT[:, 0:1], 0.5)
        sinT = work.tile([128, L], F32R if use_f32r else F32, tag="sinT")
        nc.vector.memset(sinT[:, 0:1], 0.5)
        _emit_mains(nc, consts, pools, use_f32r, cosT, sinT)
        return

    # ---- kx (turns) + prefix sum, pipelined in halves ----
    kxps = kxp.tile([128, L], F32, tag="kxps")
    ks = work.tile([128, L], F32, tag="ks")
    nc.tensor.matmul(kxps[:, 0:512], lhsT=omb, rhs=xt[:, 0:512],
                     start=True, stop=True)
    nc.vector.tensor_tensor_scan(ks[:, 0:512], kxps[:, 0:512], zeros_t,
                                 0.0, op0=OP.add, op1=OP.add)
    nc.tensor.matmul(kxps[:, 512:1024], lhsT=omb, rhs=xt[:, 512:1024],
                     start=True, stop=True)
    nc.vector.tensor_tensor_scan(ks[:, 512:1024], kxps[:, 512:1024],
                                 zeros_t, ks[:, 511:512],
                                 op0=OP.add, op1=OP.add)

    # ---- range reduction: frac = ks - round(ks) in [-0.5, 0.5] ----
    rnd = work.tile([128, L], F32, tag="rnd")
    nc.vector.tensor_scalar(rnd, ks, C_MAGIC, C_MAGIC,
                            op0=OP.add, op1=OP.subtract)
    frac = work.tile([128, L], F32, tag="frac")
    nc.vector.tensor_tensor(frac, ks, rnd, op=OP.subtract)
    # |frac| on DVE so it overlaps the ACT sin activation
    af = work.tile([128, L], F32, tag="af")
    nc.vector.scalar_tensor_tensor(af, frac, -1.0, frac,
                                   op0=OP.mult, op1=OP.max)

    # ---- trig: sin = sin(2pi f) first (no abs dep), then cos ----
    mmdt = F32R if use_f32r else F32
    sinT = work.tile([128, L], mmdt, tag="sinT")
    nc.scalar.activation(sinT, frac, AF.Sin, scale=TWO_PI, bias=zero_bias)
    cosT = work.tile([128, L], mmdt, tag="cosT")
    nc.scalar.activation(cosT, af, AF.Sin, scale=-TWO_PI, bias=halfpi_bias)

    if variant == "head":
        out = consts[-1]
        nc.sync.dma_start(out=out[0:128, :],
                          in_=cosT[:, 0:D].bitcast(F32))
        nc.scalar.dma_start(out=out[128:256, :],
                            in_=sinT[:, 0:D].bitcast(F32))
        return

    _emit_mains(nc, consts, pools, use_f32r, cosT, sinT)


def _emit_mains(nc, consts, pools, use_f32r, cosT, sinT):
    (zero_bias, halfpi_bias, zeros_t, xo, uvc_t, out) = consts
    work, kxp, ops = pools
    nmm = DP if use_f32r else D
    rhs_cos = uvc_t[:, 0:nmm]
    rhs_sin = uvc_t[:, DP:DP + nmm]
    crowB = (uvc_t[:, 2 * DP:WU].bitcast(F32)
             .rearrange("p (j d) -> p j d", j=2))
    # ---- output matmuls + fused (+const) eviction + DMA ----
    for b in range(BPC):
        pp = slice(b * M, (b + 1) * M)
        for h in range(2):
            obuf = work.tile([128, N_CHUNKS // 2, D], F32, tag=f"obuf{b}{h}")
            for i in range(0, N_CHUNKS // 2, 2):
                ic = h * (N_CHUNKS // 2) + i
                ps = ops.tile([128, 2, nmm], F32, tag="ops")
                for j in (0, 1):
                    lsl = slice((ic + j) * 128, (ic + j + 1) * 128)
                    nc.tensor.matmul(
                        ps[:, j, :], lhsT=cosT[pp, lsl],
                        rhs=rhs_cos[pp, :], start=True, stop=False)
                    nc.tensor.matmul(
                        ps[:, j, :], lhsT=sinT[pp, lsl],
                        rhs=rhs_sin[pp, :], start=False, stop=True)
                nc.vector.scalar_tensor_tensor(
                    obuf[:, i:i + 2, :], ps[:, :, 0:D], 0.0, crowB,
                    op0=OP.add, op1=OP.add,
                )
            eng = nc.sync if b == 0 else nc.scalar
            r0 = b * L + h * (L // 2)
            eng.dma_start(
                out=out[r0:r0 + L // 2, :].rearrange("(i p) d -> p i d", p=128),
                in_=obuf,
            )


def build_nc(use_f32r=True, reps=1, wfix=True, loop_k=0, variant="full"):
    nc = bass.Bass()
    udt = F32R if use_f32r else F32
    xom = nc.declare_dram_parameter("xom", [2 * BPC, WX], F32, isOutput=False)
    uvc = nc.declare_dram_parameter("uvc", [128, WU], udt, isOutput=False)
    out = nc.declare_dram_parameter("out", [LROWS, D], F32, isOutput=True)

    with PatchedTileContext(nc) as tc:
        with (
            tc.tile_pool(name="singles", bufs=1) as singles,
            tc.tile_pool(name="work", bufs=2) as work,
            tc.tile_pool(name="kxp", bufs=2, space="PSUM") as kxp,
            tc.tile_pool(name="ops", bufs=4, space="PSUM") as ops,
        ):
            zero_bias = singles.tile([128, 1], F32)
            nc.gpsimd.memset(zero_bias, 0.0)
            halfpi_bias = singles.tile([128, 1], F32)
            nc.gpsimd.memset(halfpi_bias, HALF_PI)
            zeros_t = singles.tile([128, 512], F32)
            nc.gpsimd.memset(zeros_t, 0.0)
            dummy = singles.tile([128, 1], F32)
            nc.scalar.activation(dummy, zero_bias, AF.Sin,
                                 scale=1.0, bias=zero_bias)

            xo = singles.tile([2 * BPC, WX], F32)
            nc.sync.dma_start(out=xo, in_=xom[:, :])
            uvc_t = singles.tile([128, WU], udt)
            nc.sync.dma_start(out=uvc_t, in_=uvc[:, :])

            consts = (zero_bias, halfpi_bias, zeros_t, xo, uvc_t, out)
            pools = (work, kxp, ops)
            if loop_k:
                with tc.For_i(0, loop_k, 1) as _i:
                    _emit_rep(nc, consts, pools, use_f32r, 0, variant)
            else:
                for rep in range(reps):
                    _emit_rep(nc, consts, pools, use_f32r, rep, variant)
    if wfix:
        _split_multiwait(nc)
    return nc


def host_prep(input, z0, om, S):
    """Precompute the tiny parameter-derived tensors (all O(D*M))."""
    input = np.ascontiguousarray(input, dtype=np.float32)
    z0 = np.asarray(z0, dtype=np.float32)
    om = np.asarray(om, dtype=np.float32)
    S = np.asarray(S, dtype=np.float32)

    om_t = (om.astype(np.float64) / (2.0 * np.pi)).astype(np.float32)  # turns
    w0 = np.linalg.solve(S.astype(np.float64), z0.astype(np.float64))
    a = w0[1::2]
    bb = w0[2::2]
    Sd = S.astype(np.float64)
    U = (a * Sd[:, 1::2] + bb * Sd[:, 2::2])          # [D, M]
    V = (a * Sd[:, 2::2] - bb * Sd[:, 1::2])          # [D, M]
    crow = (w0[0] * Sd[:, 0]).astype(np.float32)

    uvcp = np.zeros((128, WU), dtype=np.float32)
    uvcp[0:M, 0:D] = U.T.astype(np.float32)
    uvcp[M:2 * M, 0:D] = U.T.astype(np.float32)
    uvcp[0:M, DP:DP + D] = V.T.astype(np.float32)
    uvcp[M:2 * M, DP:DP + D] = V.T.astype(np.float32)
    uvcp[:, 2 * DP:2 * DP + D] = crow
    uvcp[:, 2 * DP + D:WU] = crow

    omblk = np.zeros((2 * BPC, 128), dtype=np.float32)
    omblk[0:2, 0:M] = om_t.T
    omblk[2:4, M:2 * M] = om_t.T

    xoms = []
    for c in range(N_CORES):
        xc = input[c * BPC:(c + 1) * BPC]            # [2, L, 2]
        xo = np.zeros((2 * BPC, WX), dtype=np.float32)
        xo[:, 0:L] = xc.transpose(0, 2, 1).reshape(2 * BPC, L)
        xo[:, L:WX] = omblk
        xoms.append(xo)
    return xoms, uvcp


def kernel(input, z0, om, S):
    from concourse.bass_utils import run_bass_kernel_spmd

    xoms, uvcp = host_prep(input, z0, om, S)
    nc = build_nc()
    in_maps = [{"xom": xoms[c], "uvc": uvcp} for c in range(N_CORES)]
    res = run_bass_kernel_spmd(nc, in_maps, list(range(N_CORES)))
    outputs = np.empty((B, L, D), dtype=np.float32)
    for c in range(N_CORES):
        outputs[c * BPC:(c + 1) * BPC] = res.results[c]["out"].reshape(BPC, L, D)
    z_final = outputs[:, -1, :].copy()
    return outputs, z_final
